# revision 11
# baseline (speedup 1.0000x reference)
"""BinaryLinear (sign-binarized weight linear layer) on 8 Trainium2 NeuronCores.

y[b,s,o] = sum_i x[b,s,i] * (scale[o] * sign(w[o,i])) + bias[o]
  with scale[o] = mean_i |w[o,i]|

Sharding: data-parallel over the batch dim (8 batches -> 8 cores); w/bias
replicated. Per core (m = sequence rows, o = out features, k = in features):

  - x f32 -> bf16 via SWDGE cast-DMA (DRAM->DRAM), then XBAR DMA-transposed
    into a fully SBUF-resident x^T [k-part, m] in s-contiguous 1 MB slabs
  - w binarized on-chip: ACT Sign -> bf16 B rows in SBUF, DVE abs-row-sum ->
    scale; B^T via XBAR transpose directly from SBUF (no DRAM round trip)
  - TensorE: yT[o,m] = B^T.T @ x^T accumulated over k in PSUM (bf16 inputs,
    f32 accumulate); DVE fuses psum*scale[o]+bias[o] on PSUM eviction
  - ALL XBAR transposes issue on the sync queue ONLY: concurrent transposes
    on both HWDGE queues corrupt each other (measured), and the scheduler
    serializes a transpose against every other in-flight DMA anyway
  - phases ordered so weight prep & x^T build hide under the first n-column
    of matmuls; B^T tiles for the early o-rows are streamed twice to allow it

Host side only shards inputs and transposes yT shards back into y.
"""

import numpy as np

B_DIM = 8
S_DIM = 2048
IN_F = 4096
OUT_F = 4096
P = 128
N_CORES = 8
N_TILE = 512

_BUILT = None


def _build_nc(s_dim=S_DIM, in_f=IN_F, out_f=OUT_F):
    from contextlib import ExitStack

    import concourse.mybir as mybir
    import concourse.tile as tile
    from concourse import bacc
    from concourse.bass import ts

    f32 = mybir.dt.float32
    bf16 = mybir.dt.bfloat16

    NCH = s_dim // N_TILE  # n chunks (moving-dim tiles of 512)
    PO = out_f // P  # o blocks (output-partition tiles of 128)
    KT = in_f // P  # contraction subtiles of 128
    HALF = in_f // 2
    NXU = 8  # x cast units (256 rows each)
    ROWS_U = s_dim // NXU
    NSLAB = s_dim // P  # x transpose slabs (128 rows each)
    # phase A runs n=0 for the first A_N o-blocks so the x^T build (cast +
    # transpose DMAs) can finish in their shadow
    A_N = min(16, PO) if NCH > 1 else PO

    nc = bacc.Bacc(None, target_bir_lowering=False, debug=False)
    with tile.TileContext(nc) as tc:
        x_d = nc.dram_tensor("x", (s_dim, in_f), f32, kind="ExternalInput")
        w_d = nc.dram_tensor("w", (out_f, in_f), f32, kind="ExternalInput")
        b_d = nc.dram_tensor("bias", (out_f,), f32, kind="ExternalInput")
        yT_d = nc.dram_tensor("yT", (out_f, s_dim), f32, kind="ExternalOutput")

        with ExitStack() as ctx:
            dram = ctx.enter_context(tc.tile_pool(name="dram", bufs=1, space="DRAM"))
            xbf_d = dram.tile((s_dim, in_f), bf16)
            # B rows for the phase-A blocks only: phase B re-streams their
            # B^T after the SBUF b tiles have been recycled
            bw_d = dram.tile((A_N * P, in_f), bf16)
            bw3 = bw_d[:, :].rearrange("o (kt ki) -> o kt ki", ki=P)
            yT3 = yT_d[:, :].rearrange("(po pi) s -> pi po s", pi=P)

            const = ctx.enter_context(tc.tile_pool(name="const", bufs=1))
            xT = const.tile([P, NCH, KT, N_TILE], bf16)  # resident x^T
            scale_sb = const.tile([P, PO], f32)
            bias_sb = const.tile([P, PO], f32)
            nc.scalar.dma_start(bias_sb[:], b_d[:].rearrange("(po pi) -> pi po", pi=P))

            wpool = ctx.enter_context(tc.tile_pool(name="wpool", bufs=4))
            bpool = ctx.enter_context(tc.tile_pool(name="bpool", bufs=2))
            btpool = ctx.enter_context(tc.tile_pool(name="btpool", bufs=2))
            scpool = ctx.enter_context(tc.tile_pool(name="scpool", bufs=2))
            opool = ctx.enter_context(tc.tile_pool(name="opool", bufs=7))
            psum = ctx.enter_context(tc.tile_pool(name="psum", bufs=6, space="PSUM"))

            def cast_x_unit(u):
                # cast rows f32->bf16 (SWDGE, DRAM->DRAM); casts serialize on
                # the gpsimd queue, so issue them all up front
                nc.gpsimd.dma_start(xbf_d[ts(u, ROWS_U), :], x_d[ts(u, ROWS_U), :])

            next_slab = 0

            def build_x_slab():
                # XBAR-transpose one 128-row slab of cast x into the resident
                # x^T ([128 rows, in_f] is contiguous in DRAM and reads at
                # full rate; the k-split alternative generates strided
                # descriptors and runs ~3x slower)
                nonlocal next_slab
                if next_slab >= NSLAB:
                    return
                g = next_slab
                next_slab += 1
                c, s0 = divmod(g * P, N_TILE)
                nc.sync.dma_start_transpose(
                    xT[:, c, :, s0 : s0 + P],
                    xbf_d[ts(g, P), :],
                )

            # W prep is software-pipelined at emission time
            w_tiles = {}

            def load_w(m):
                halves = []
                for h in range(2):
                    w_sb = wpool.tile([P, HALF], f32, tag="w", name=f"w_{m}_{h}")
                    nc.scalar.dma_start(w_sb[:], w_d[ts(m, P), ts(h, HALF)])
                    halves.append(w_sb)
                w_tiles[m] = halves

            b_tiles = {}

            def process_w(m):
                # sign -> bf16 B rows in SBUF (ACT), |w| row sums -> scale (DVE)
                b_sb = bpool.tile([P, in_f], bf16)
                sc2 = scpool.tile([P, 2], f32)
                for h in range(2):
                    w_sb = w_tiles[m][h]
                    nc.scalar.sign(b_sb[:, ts(h, HALF)], w_sb[:])
                    nc.vector.tensor_reduce(
                        sc2[:, h : h + 1],
                        w_sb[:],
                        axis=mybir.AxisListType.X,
                        op=mybir.AluOpType.add,
                        apply_absolute_value=True,
                    )
                del w_tiles[m]
                nc.vector.tensor_reduce(
                    scale_sb[:, m : m + 1],
                    sc2[:],
                    axis=mybir.AxisListType.X,
                    op=mybir.AluOpType.add,
                )
                nc.vector.tensor_scalar_mul(
                    scale_sb[:, m : m + 1], scale_sb[:, m : m + 1], 1.0 / in_f
                )
                if m < A_N and NCH > 1:
                    nc.scalar.dma_start(bw_d[ts(m, P), :], b_sb[:])
                b_tiles[m] = b_sb

            def load_bt(m, from_dram=False):
                # B^T via XBAR transpose straight from the SBUF-resident sign
                # output (sync queue only -- see module docstring); phase B
                # re-streams from the DRAM spill instead
                bt = btpool.tile([P, KT, P], bf16)
                if from_dram:
                    nc.sync.dma_start_transpose(bt[:], bw3[ts(m, P)])
                else:
                    b3 = b_tiles[m][:].rearrange("o (kt ki) -> o kt ki", ki=P)
                    nc.sync.dma_start_transpose(bt[:], b3)
                    del b_tiles[m]
                return bt

            def mm_block(bt, m, n):
                ps = psum.tile([P, N_TILE], f32, name="ps")
                for kt in range(KT):
                    nc.tensor.matmul(
                        ps[:],
                        bt[:, kt, :],
                        xT[:, n, kt, :],
                        start=(kt == 0),
                        stop=(kt == KT - 1),
                    )
                ob = opool.tile([P, N_TILE], f32)
                nc.vector.tensor_scalar(
                    ob[:],
                    ps[:],
                    scale_sb[:, m : m + 1],
                    bias_sb[:, m : m + 1],
                    op0=mybir.AluOpType.mult,
                    op1=mybir.AluOpType.add,
                )
                nc.scalar.dma_start(yT3[:, m, ts(n, N_TILE)], ob[:])

            # x pipeline first: casts chain on gpsimd; transposes are emitted
            # interleaved with the bt transposes below so they batch on sync
            for u in range(NXU):
                cast_x_unit(u)

            # W-prep runs `next_proc` blocks ahead of consumption
            load_w(0)
            load_w(1)
            process_w(0)
            next_proc = 1

            def advance_prep():
                nonlocal next_proc
                if next_proc < PO:
                    if next_proc + 1 < PO:
                        load_w(next_proc + 1)
                    process_w(next_proc)
                    next_proc += 1

            # chunk-0 slabs first so the first matmul block can start ASAP
            for _ in range(N_TILE // P):
                build_x_slab()
            # phase A: n=0 for the first A_N o-blocks while x^T builds; one
            # x-slab transpose per block -- more would starve the bt stream
            # (sync FIFO head-of-line blocks on the slab's pending cast)
            for m in range(A_N):
                bt = load_bt(m)
                build_x_slab()
                advance_prep()
                mm_block(bt, m, 0)
            # phase B: remaining n for those o-blocks (B^T tiles re-streamed)
            if NCH > 1:
                for m in range(A_N):
                    bt = load_bt(m, from_dram=True)
                    build_x_slab()
                    advance_prep()
                    for n in range(1, NCH):
                        mm_block(bt, m, n)
            # phase C: the rest, n inner
            for m in range(A_N, PO):
                bt = load_bt(m)
                build_x_slab()
                advance_prep()
                for n in range(NCH):
                    mm_block(bt, m, n)
    nc.finalize()
    return nc


def _get_nc():
    global _BUILT
    if _BUILT is None:
        _BUILT = _build_nc()
    return _BUILT


def kernel(x, weight, bias):
    from concourse.bass_utils import run_bass_kernel_spmd

    x = np.asarray(x, dtype=np.float32)
    weight = np.asarray(weight, dtype=np.float32)
    bias = np.asarray(bias, dtype=np.float32)
    assert x.shape == (B_DIM, S_DIM, IN_F), x.shape

    nc = _get_nc()
    in_maps = [
        {"x": np.ascontiguousarray(x[b]), "w": weight, "bias": bias}
        for b in range(N_CORES)
    ]
    res = run_bass_kernel_spmd(nc, in_maps, core_ids=list(range(N_CORES)))
    out = np.empty((B_DIM, S_DIM, OUT_F), dtype=np.float32)
    for b in range(N_CORES):
        out[b] = res.results[b]["yT"].T
    return out


# revision 15
# speedup vs baseline: 1.1160x; 1.1160x over previous
"""BinaryLinear (sign-binarized weight linear layer) on 8 Trainium2 NeuronCores.

y[b,s,o] = sum_i x[b,s,i] * (scale[o] * sign(w[o,i])) + bias[o]
  with scale[o] = mean_i |w[o,i]|

Sharding: data-parallel over the batch dim (8 batches -> 8 cores); w/bias
replicated. Per core (m = sequence rows, o = out features, k = in features):

  - x f32 -> bf16 via SWDGE cast-DMA (DRAM->DRAM), then XBAR DMA-transposed
    into a fully SBUF-resident x^T [k-part, m] in s-contiguous 1 MB slabs
  - w binarized on-chip: ACT Sign -> bf16 B rows in SBUF, DVE abs-row-sum ->
    scale; B^T via XBAR transpose directly from SBUF (no DRAM round trip)
  - TensorE: yT[o,m] = B^T.T @ x^T accumulated over k in PSUM (bf16 inputs,
    f32 accumulate); DVE fuses psum*scale[o]+bias[o] on PSUM eviction
  - ALL XBAR transposes issue on the sync queue ONLY: concurrent transposes
    on both HWDGE queues corrupt each other (measured), and the scheduler
    serializes a transpose against every other in-flight DMA anyway
  - phases ordered so weight prep & x^T build hide under the first n-column
    of matmuls; B^T tiles for the early o-rows are streamed twice to allow it

Host side only shards inputs and transposes yT shards back into y.
"""

import numpy as np

B_DIM = 8
S_DIM = 2048
IN_F = 4096
OUT_F = 4096
P = 128
N_CORES = 8
N_TILE = 512

_BUILT = None


def _build_nc(s_dim=S_DIM, in_f=IN_F, out_f=OUT_F):
    from contextlib import ExitStack

    import concourse.mybir as mybir
    import concourse.tile as tile
    from concourse import bacc
    from concourse.bass import ts

    f32 = mybir.dt.float32
    bf16 = mybir.dt.bfloat16

    NCH = s_dim // N_TILE  # n chunks (moving-dim tiles of 512)
    PO = out_f // P  # o blocks (output-partition tiles of 128)
    KT = in_f // P  # contraction subtiles of 128
    HALF = in_f // 2
    NXU = 8  # x cast units (256 rows each)
    ROWS_U = s_dim // NXU
    NSLAB = s_dim // P  # x transpose slabs (128 rows each)
    # graduated warm-up: phase p runs the first p+1 n-chunks for a few
    # o-blocks while the next x^T chunk builds; each successive phase has
    # more PE time per B^T tile, which is what buys DMA slack for the x
    # build (an n=0-only phase has none).  (o_start, o_end, n_count)
    if NCH > 1:
        GRAD = [(0, 4, 1), (4, 8, 2), (8, 11, 3), (11, PO, NCH)]
        # cleanup: finish the n-chunks the warm-up phases skipped
        CLEAN = [(0, 4, 1), (4, 8, 2), (8, 11, 3)]
        SPILL_N = 11  # o-blocks whose B rows spill to DRAM for re-streaming
    else:
        GRAD = [(0, PO, 1)]
        CLEAN = []
        SPILL_N = 0

    nc = bacc.Bacc(None, target_bir_lowering=False, debug=False)
    with tile.TileContext(nc) as tc:
        x_d = nc.dram_tensor("x", (s_dim, in_f), f32, kind="ExternalInput")
        w_d = nc.dram_tensor("w", (out_f, in_f), f32, kind="ExternalInput")
        b_d = nc.dram_tensor("bias", (out_f,), f32, kind="ExternalInput")
        yT_d = nc.dram_tensor("yT", (out_f, s_dim), f32, kind="ExternalOutput")

        with ExitStack() as ctx:
            dram = ctx.enter_context(tc.tile_pool(name="dram", bufs=1, space="DRAM"))
            xbf_d = dram.tile((s_dim, in_f), bf16)
            # B rows for the warm-up blocks only: cleanup re-streams their
            # B^T after the SBUF b tiles have been recycled
            bw_d = dram.tile((max(SPILL_N, 1) * P, in_f), bf16)
            bw3 = bw_d[:, :].rearrange("o (kt ki) -> o kt ki", ki=P)
            yT3 = yT_d[:, :].rearrange("(po pi) s -> pi po s", pi=P)

            const = ctx.enter_context(tc.tile_pool(name="const", bufs=1))
            xT = const.tile([P, NCH, KT, N_TILE], bf16)  # resident x^T
            scale_sb = const.tile([P, PO], f32)
            bias_sb = const.tile([P, PO], f32)
            nc.scalar.dma_start(bias_sb[:], b_d[:].rearrange("(po pi) -> pi po", pi=P))

            wpool = ctx.enter_context(tc.tile_pool(name="wpool", bufs=4))
            bpool = ctx.enter_context(tc.tile_pool(name="bpool", bufs=2))
            btpool = ctx.enter_context(tc.tile_pool(name="btpool", bufs=2))
            scpool = ctx.enter_context(tc.tile_pool(name="scpool", bufs=2))
            opool = ctx.enter_context(tc.tile_pool(name="opool", bufs=7))
            psum = ctx.enter_context(tc.tile_pool(name="psum", bufs=6, space="PSUM"))

            def cast_x_unit(u):
                # cast rows f32->bf16 (SWDGE, DRAM->DRAM); casts serialize on
                # the gpsimd queue, so issue them all up front
                nc.gpsimd.dma_start(xbf_d[ts(u, ROWS_U), :], x_d[ts(u, ROWS_U), :])

            next_slab = 0

            def build_x_slab():
                # XBAR-transpose one 128-row slab of cast x into the resident
                # x^T ([128 rows, in_f] is contiguous in DRAM and reads at
                # full rate; the k-split alternative generates strided
                # descriptors and runs ~3x slower)
                nonlocal next_slab
                if next_slab >= NSLAB:
                    return
                g = next_slab
                next_slab += 1
                c, s0 = divmod(g * P, N_TILE)
                nc.sync.dma_start_transpose(
                    xT[:, c, :, s0 : s0 + P],
                    xbf_d[ts(g, P), :],
                )

            # W prep is software-pipelined at emission time
            w_tiles = {}

            def load_w(m):
                halves = []
                for h in range(2):
                    w_sb = wpool.tile([P, HALF], f32, tag="w", name=f"w_{m}_{h}")
                    nc.scalar.dma_start(w_sb[:], w_d[ts(m, P), ts(h, HALF)])
                    halves.append(w_sb)
                w_tiles[m] = halves

            b_tiles = {}

            def process_w(m):
                # sign -> bf16 B rows in SBUF (ACT), |w| row sums -> scale (DVE)
                b_sb = bpool.tile([P, in_f], bf16)
                sc2 = scpool.tile([P, 2], f32)
                for h in range(2):
                    w_sb = w_tiles[m][h]
                    nc.scalar.sign(b_sb[:, ts(h, HALF)], w_sb[:])
                    nc.vector.tensor_reduce(
                        sc2[:, h : h + 1],
                        w_sb[:],
                        axis=mybir.AxisListType.X,
                        op=mybir.AluOpType.add,
                        apply_absolute_value=True,
                    )
                del w_tiles[m]
                nc.vector.tensor_reduce(
                    scale_sb[:, m : m + 1],
                    sc2[:],
                    axis=mybir.AxisListType.X,
                    op=mybir.AluOpType.add,
                )
                nc.vector.tensor_scalar_mul(
                    scale_sb[:, m : m + 1], scale_sb[:, m : m + 1], 1.0 / in_f
                )
                if m < SPILL_N:
                    nc.scalar.dma_start(bw_d[ts(m, P), :], b_sb[:])
                b_tiles[m] = b_sb

            def load_bt(m, from_dram=False):
                # B^T via XBAR transpose straight from the SBUF-resident sign
                # output (sync queue only -- see module docstring); phase B
                # re-streams from the DRAM spill instead
                bt = btpool.tile([P, KT, P], bf16)
                if from_dram:
                    nc.sync.dma_start_transpose(bt[:], bw3[ts(m, P)])
                else:
                    b3 = b_tiles[m][:].rearrange("o (kt ki) -> o kt ki", ki=P)
                    nc.sync.dma_start_transpose(bt[:], b3)
                    del b_tiles[m]
                return bt

            def mm_block(bt, m, n):
                ps = psum.tile([P, N_TILE], f32, name="ps")
                for kt in range(KT):
                    nc.tensor.matmul(
                        ps[:],
                        bt[:, kt, :],
                        xT[:, n, kt, :],
                        start=(kt == 0),
                        stop=(kt == KT - 1),
                    )
                ob = opool.tile([P, N_TILE], f32)
                nc.vector.tensor_scalar(
                    ob[:],
                    ps[:],
                    scale_sb[:, m : m + 1],
                    bias_sb[:, m : m + 1],
                    op0=mybir.AluOpType.mult,
                    op1=mybir.AluOpType.add,
                )
                nc.scalar.dma_start(yT3[:, m, ts(n, N_TILE)], ob[:])

            # w0/w1 loads first (before the casts grab HBM), then the x casts
            load_w(0)
            load_w(1)
            for u in range(NXU):
                cast_x_unit(u)
            process_w(0)
            next_proc = 1

            def advance_prep():
                nonlocal next_proc
                if next_proc < PO:
                    if next_proc + 1 < PO:
                        load_w(next_proc + 1)
                    process_w(next_proc)
                    next_proc += 1

            # chunk-0 slabs first so the first matmul block can start ASAP
            for _ in range(N_TILE // P):
                build_x_slab()
            # graduated warm-up (see GRAD comment above); one x-slab
            # transpose per block keeps the next chunk building without
            # starving the bt stream
            for o0, o1, nct in GRAD:
                for m in range(o0, o1):
                    bt = load_bt(m)
                    build_x_slab()
                    advance_prep()
                    for n in range(nct):
                        mm_block(bt, m, n)
            # cleanup: the n-chunks the warm-up skipped (B^T re-streamed
            # from the DRAM spill)
            for o0, o1, nct in CLEAN:
                for m in range(o0, o1):
                    bt = load_bt(m, from_dram=True)
                    advance_prep()
                    for n in range(nct, NCH):
                        mm_block(bt, m, n)
    nc.finalize()
    return nc


def _get_nc():
    global _BUILT
    if _BUILT is None:
        _BUILT = _build_nc()
    return _BUILT


def kernel(x, weight, bias):
    from concourse.bass_utils import run_bass_kernel_spmd

    x = np.asarray(x, dtype=np.float32)
    weight = np.asarray(weight, dtype=np.float32)
    bias = np.asarray(bias, dtype=np.float32)
    assert x.shape == (B_DIM, S_DIM, IN_F), x.shape

    nc = _get_nc()
    in_maps = [
        {"x": np.ascontiguousarray(x[b]), "w": weight, "bias": bias}
        for b in range(N_CORES)
    ]
    res = run_bass_kernel_spmd(nc, in_maps, core_ids=list(range(N_CORES)))
    out = np.empty((B_DIM, S_DIM, OUT_F), dtype=np.float32)
    for b in range(N_CORES):
        out[b] = res.results[b]["yT"].T
    return out


# revision 23
# speedup vs baseline: 1.1441x; 1.0252x over previous
"""BinaryLinear (sign-binarized weight linear layer) on 8 Trainium2 NeuronCores.

y[b,s,o] = sum_i x[b,s,i] * (scale[o] * sign(w[o,i])) + bias[o]
  with scale[o] = mean_i |w[o,i]|

Sharding: data-parallel over the batch dim (8 batches -> 8 cores); w/bias
replicated. Per core (m = sequence rows, o = out features, k = in features):

  - x and w stream in as bf16 via SWDGE cast-DMA (DRAM f32 -> SBUF bf16),
    halving their HBM traffic; sign/scale tolerate the bf16 w rounding
  - x^T built by XBAR DMA-transpose from the SBUF slabs; w binarized by ACT
    Sign in SBUF, then B^T XBAR-transposed from SBUF (no DRAM round trips)
  - TensorE: yT[o,m] = B^T.T @ x^T accumulated over k in PSUM (bf16 inputs,
    f32 accumulate); DVE fuses psum*scale[o]+bias[o] on PSUM eviction,
    emitting bf16 (host upcasts; the 0.4% rounding is well inside tolerance)
  - ALL XBAR transposes issue on the sync queue ONLY: concurrent transposes
    on both HWDGE queues corrupt each other (measured), and the scheduler
    serializes a transpose against every other in-flight DMA regardless
  - graduated warm-up: early o-blocks run only the n-chunks already built,
    giving the x^T build DMA slack that an all-chunks start would not have;
    their missing n-chunks run at the end from a DRAM spill of B
"""

import numpy as np

B_DIM = 8
S_DIM = 2048
IN_F = 4096
OUT_F = 4096
P = 128
N_CORES = 8
N_TILE = 512

_BUILT = None


def _build_nc(s_dim=S_DIM, in_f=IN_F, out_f=OUT_F):
    from contextlib import ExitStack

    import concourse.mybir as mybir
    import concourse.tile as tile
    from concourse import bacc
    from concourse.bass import ts

    f32 = mybir.dt.float32
    bf16 = mybir.dt.bfloat16

    NCH = s_dim // N_TILE  # n chunks (moving-dim tiles of 512)
    PO = out_f // P  # o blocks (output-partition tiles of 128)
    KT = in_f // P  # contraction subtiles of 128
    NSLAB = s_dim // P  # x slabs (128 rows each)
    SL_CH = N_TILE // P  # slabs per chunk
    # graduated warm-up phases: (o_start, o_end, n_count).
    # ORDERING INVARIANT: the matmul's strided rhs read of x^T is NOT
    # dependency-tracked against the slab transposes (observed race), so
    # correctness relies on sync-queue FIFO: every slab transpose of chunk c
    # must be EMITTED before the bt transpose of the first block reading c
    # (the matmul's dep on its contiguous bt read is real, and the bt
    # transpose completes only after all earlier sync-queue transposes).
    # With 2 slabs emitted per block ahead of the bt, slabs available before
    # bt[m] = 2m+6; first readers below need 8/12/16 at m=2/5/9.
    if NCH > 1:
        GRAD = [(0, 2, 1), (2, 5, 2), (5, 9, 3), (9, PO, NCH)]
        CLEAN = [(0, 2, 1), (2, 5, 2), (5, 9, 3)]
        SPILL_N = 9
    else:
        GRAD = [(0, PO, 1)]
        CLEAN = []
        SPILL_N = 0

    nc = bacc.Bacc(None, target_bir_lowering=False, debug=False)
    with tile.TileContext(nc) as tc:
        x_d = nc.dram_tensor("x", (s_dim, in_f), f32, kind="ExternalInput")
        w_d = nc.dram_tensor("w", (out_f, in_f), f32, kind="ExternalInput")
        b_d = nc.dram_tensor("bias", (out_f,), f32, kind="ExternalInput")
        yT_d = nc.dram_tensor("yT", (out_f, s_dim), bf16, kind="ExternalOutput")

        with ExitStack() as ctx:
            dram = ctx.enter_context(tc.tile_pool(name="dram", bufs=1, space="DRAM"))
            # B rows for the warm-up blocks only: cleanup re-streams their
            # B^T after the SBUF b tiles have been recycled
            bw_d = dram.tile((max(SPILL_N, 1) * P, in_f), bf16)
            bw3 = bw_d[:, :].rearrange("o (kt ki) -> o kt ki", ki=P)
            yT3 = yT_d[:, :].rearrange("(po pi) s -> pi po s", pi=P)

            const = ctx.enter_context(tc.tile_pool(name="const", bufs=1))
            # slab-major layout: each XBAR transpose writes one fully
            # contiguous [P, KT, P] block (a strided per-slab footprint is
            # mis-modeled by the dependency tracker -> matmuls race the
            # transpose); the matmul reads across slabs with a 3D AP
            xT = const.tile([P, NSLAB, KT, P], bf16)  # resident x^T
            scale_sb = const.tile([P, PO], f32)
            bias_sb = const.tile([P, PO], f32)
            nc.scalar.dma_start(bias_sb[:], b_d[:].rearrange("(po pi) -> pi po", pi=P))

            wpool = ctx.enter_context(tc.tile_pool(name="wpool", bufs=2))
            bpool = ctx.enter_context(tc.tile_pool(name="bpool", bufs=2))
            xpool = ctx.enter_context(tc.tile_pool(name="xpool", bufs=3))
            btpool = ctx.enter_context(tc.tile_pool(name="btpool", bufs=2))
            opool = ctx.enter_context(tc.tile_pool(name="opool", bufs=7))
            psum = ctx.enter_context(tc.tile_pool(name="psum", bufs=6, space="PSUM"))

            # ---- x pipeline: SWDGE cast-load slab -> XBAR transpose ----
            x_tiles = {}
            next_load = 0

            def load_x_slab():
                nonlocal next_load
                if next_load >= NSLAB:
                    return
                g = next_load
                next_load += 1
                xr = xpool.tile([P, in_f], bf16, tag="xr", name=f"x_{g}")
                nc.gpsimd.dma_start(xr[:], x_d[ts(g, P), :])
                x_tiles[g] = xr

            next_slab = 0

            def build_x_slab():
                nonlocal next_slab
                if next_slab >= NSLAB:
                    return
                g = next_slab
                next_slab += 1
                nc.sync.dma_start_transpose(
                    xT[:, g, :, :],
                    x_tiles.pop(g)[:],
                )

            # ---- w pipeline: SWDGE cast-load -> ACT sign -> XBAR B^T ----
            w_tiles = {}

            def load_w(m):
                w_sb = wpool.tile([P, in_f], bf16, tag="w", name=f"w_{m}")
                nc.gpsimd.dma_start(w_sb[:], w_d[ts(m, P), :])
                w_tiles[m] = w_sb

            b_tiles = {}

            def process_w(m):
                b_sb = bpool.tile([P, in_f], bf16)
                w_sb = w_tiles.pop(m)
                nc.scalar.sign(b_sb[:], w_sb[:])
                nc.vector.tensor_reduce(
                    scale_sb[:, m : m + 1],
                    w_sb[:],
                    axis=mybir.AxisListType.X,
                    op=mybir.AluOpType.add,
                    apply_absolute_value=True,
                )
                nc.vector.tensor_scalar_mul(
                    scale_sb[:, m : m + 1], scale_sb[:, m : m + 1], 1.0 / in_f
                )
                if m < SPILL_N:
                    nc.scalar.dma_start(bw_d[ts(m, P), :], b_sb[:])
                b_tiles[m] = b_sb

            def load_bt(m, from_dram=False):
                bt = btpool.tile([P, KT, P], bf16)
                if from_dram:
                    nc.sync.dma_start_transpose(bt[:], bw3[ts(m, P)])
                else:
                    b3 = b_tiles.pop(m)[:].rearrange("o (kt ki) -> o kt ki", ki=P)
                    nc.sync.dma_start_transpose(bt[:], b3)
                return bt

            def mm_block(bt, m, n):
                ps = psum.tile([P, N_TILE], f32, name="ps")
                for kt in range(KT):
                    nc.tensor.matmul(
                        ps[:],
                        bt[:, kt, :],
                        xT[:, ts(n, SL_CH), kt, :],
                        start=(kt == 0),
                        stop=(kt == KT - 1),
                    )
                ob = opool.tile([P, N_TILE], bf16)
                nc.vector.tensor_scalar(
                    ob[:],
                    ps[:],
                    scale_sb[:, m : m + 1],
                    bias_sb[:, m : m + 1],
                    op0=mybir.AluOpType.mult,
                    op1=mybir.AluOpType.add,
                )
                nc.scalar.dma_start(yT3[:, m, ts(n, N_TILE)], ob[:])

            # ---- emission ----
            # bootstrap: w0/w1 + first six x-slab loads, then the chunk-0
            # transposes
            load_w(0)
            load_w(1)
            for _ in range(SL_CH + 2):
                load_x_slab()
            process_w(0)
            next_proc = 1

            def advance_prep():
                nonlocal next_proc
                if next_proc < PO:
                    if next_proc + 1 < PO:
                        load_w(next_proc + 1)
                    process_w(next_proc)
                    next_proc += 1

            for _ in range(SL_CH):
                build_x_slab()

            for o0, o1, nct in GRAD:
                for m in range(o0, o1):
                    # slab loads + transposes BEFORE the bt transpose: the
                    # FIFO ordering invariant above depends on this
                    load_x_slab()
                    load_x_slab()
                    build_x_slab()
                    build_x_slab()
                    bt = load_bt(m)
                    advance_prep()
                    for n in range(nct):
                        mm_block(bt, m, n)
            for o0, o1, nct in CLEAN:
                for m in range(o0, o1):
                    bt = load_bt(m, from_dram=True)
                    advance_prep()
                    for n in range(nct, NCH):
                        mm_block(bt, m, n)
    nc.finalize()
    return nc


def _get_nc():
    global _BUILT
    if _BUILT is None:
        _BUILT = _build_nc()
    return _BUILT


def kernel(x, weight, bias):
    from concourse.bass_utils import run_bass_kernel_spmd

    x = np.asarray(x, dtype=np.float32)
    weight = np.asarray(weight, dtype=np.float32)
    bias = np.asarray(bias, dtype=np.float32)
    assert x.shape == (B_DIM, S_DIM, IN_F), x.shape

    nc = _get_nc()
    in_maps = [
        {"x": np.ascontiguousarray(x[b]), "w": weight, "bias": bias}
        for b in range(N_CORES)
    ]
    res = run_bass_kernel_spmd(nc, in_maps, core_ids=list(range(N_CORES)))
    out = np.empty((B_DIM, S_DIM, OUT_F), dtype=np.float32)
    for b in range(N_CORES):
        out[b] = res.results[b]["yT"].astype(np.float32).T
    return out
